# revision 15
# baseline (speedup 1.0000x reference)
"""TRN2 Bass kernel for nn_MultiHeadAttention_42511586296095.

Reference math (B=4, S=2048, E=768, H=12, full-width per-head projections):
    q_h = x @ Wq_h + bq_h ; k_h = x @ Wk_h + bk_h ; v_h = x @ Wv_h + bv_h
    attn_h = softmax(q_h k_h^T / 8)
    out = sum_h (attn_h v_h) @ W0_h + b0

Sharding: 8 cores = 4 batches x 2 head-groups (6 heads each). Host sums the
two per-batch partials (device already accumulated over its own heads via
DRAM accum-DMA) and adds constants.

v3: algebraic folds of v2 (A = Wk Wq^T so scoresT = (x A) x^T with a
per-key bias beta; C = Wv W0 so the output projection disappears) plus a
transposed PV phase:

    P1: uT[f,j] = sum_e A[e,f] xT[e,j]            (keys j on free dim)
    P2: v[j,f]  = x C  -> SBUF bf16 [j, 772] with col 768 = 1.0
    P3 per 512-query group:
        scores:  16 chains psc = uT_jt^T xT_ig -> ACT exp (scale 1/8,
                 bias beta/8) -> bf16 strip st[key, query]  (2 PSUM banks,
                 double buffered)
        PV (transposed): per 128-query chunk qc, stationary = st[:, qc],
                 moving = v rows: yqd[q, 0:769] += st_chunk^T v_jt over all
                 16 key tiles.  Column 768 of v is 1.0, so yqd[:, 768] is
                 the softmax denominator -- no separate ones matmul.
        drain:   DVE reciprocal rcb = 1/yqd[:,768:769]; ACT Copy with
                 per-partition scale=rcb writes the normalized [128, 768]
                 bf16 output chunk; accum-DMA into DRAM across heads
                 (bypass on head 0).  Output layout is natural [S, E] --
                 the host just reshapes and upcasts, no transpose.

PSUM: tag "a" (P1/P2 chains + scores) 3 banks, tag "y" (yqd [128,772] =
2 banks each) x2 = 4 banks; 7 of 8 banks used (a3y2 measured best).  The
PV accumulation target is double buffered so query-chunk boundaries never
stall the PE, and scores chains only ever wait on an ACT drain three
chains back.

v4: ALL matmuls bf16 (f32 PSUM accumulation).  Same cycle count as f32r
(1 col/cycle, free dims 512/384/512/257), but ~half the PE datapath
energy and half the A/C weight-DMA traffic (28->14 MB/iter).  That
matters because the sustained clock is power-throttled: a P3-only
ablation (bf16-heavy) ran at the full 2.4 GHz while the f32r-heavy full
kernel sat at ~2.0 GHz.  Back-to-back, bf16 measures ~10-15% faster than
the f32r build at matched thermal state.  Numerics: bf16 operand noise
adds ~0.4% attention-weight jitter; measured rel err 8.8e-3 vs the 2e-2
gate.

Calibration (microbench, sustained 8-core): PE effective clock is
power-state dependent (2.0-2.4 GHz) + ~10-18 ns/matmul issue overhead;
matmul free dim is hard-capped at 512 (walrus s3d3_mm_num_elements, any
dtype, 3D APs too).  Stream floor 3.245M PE cycles -> 1.35 ms @2.4GHz;
this kernel measures ~1.68 ms (cool) to ~2.0 ms (throttled).
"""

import numpy as np

import concourse.bass as bass
import concourse.mybir as mybir
import concourse.tile as tile
from concourse import bacc
from concourse.bass_utils import run_bass_kernel_spmd

F32 = mybir.dt.float32
F32R = mybir.dt.float32r
BF16 = mybir.dt.bfloat16
EXP = mybir.ActivationFunctionType.Exp
COPY = mybir.ActivationFunctionType.Copy
ADD = mybir.AluOpType.add

B, S, E, H = 4, 2048, 768, 12
HPC = 6          # heads per core
EC = E // 128    # 6 chunks of the feature dim
JT = S // 128    # 16 key tiles
IG = 4           # query groups
IGW = S // IG    # 512 queries per group
FH = 384         # v-feature half width (2 halves of E)
QC = S // 128    # 16 query chunks (PV/output granularity)
VW = 772         # v_sb row width: 768 features + ones col at 768 + pad

_CACHED_NC = None


def _round_f32r(x: np.ndarray) -> np.ndarray:
    """Round fp32 to the hw f32r format: 11 explicit mantissa bits, RNE."""
    b = np.ascontiguousarray(x, dtype=np.float32).view(np.uint32).astype(np.uint64)
    shift = 12
    half = np.uint64(1 << (shift - 1))
    mask = np.uint64((1 << shift) - 1)
    r = (b + half) & ~mask
    tie = (b & mask) == half
    r[tie] = (b[tie] & ~mask) + (
        ((b[tie] >> np.uint64(shift)) & np.uint64(1)) << np.uint64(shift)
    )
    return r.astype(np.uint32).view(np.float32).reshape(x.shape)


def _chunked(a: np.ndarray) -> np.ndarray:
    """[E, N] -> SBUF layout [128, EC, N] with e = ec*128 + p."""
    ec = a.shape[0] // 128
    return np.ascontiguousarray(a.reshape(ec, 128, -1).transpose(1, 0, 2))


def _build_nc(hpc=HPC, loop=None, skip_p12=False, skip_p3=False, tiny_out=False, tiny_in=False, out_bf16=True, psa_bufs=3, psy_bufs=2):
    nc = bacc.Bacc("TRN2", target_bir_lowering=False, debug=False, num_devices=8)

    xT_d = nc.dram_tensor("xT", [128, EC, S], BF16, kind="ExternalInput")
    A_d = nc.dram_tensor("A", [HPC, EC, 128, EC, 128], BF16, kind="ExternalInput")
    C_d = nc.dram_tensor("C", [HPC, 2, 128, EC, FH], BF16, kind="ExternalInput")
    beta_d = nc.dram_tensor("beta8", [HPC, 128, JT], F32, kind="ExternalInput")
    ODT = BF16 if out_bf16 else F32
    out_d = nc.dram_tensor("out", [QC, 128, E], ODT, kind="ExternalOutput")

    out_chain = {}

    with tile.TileContext(nc) as tc:
        with (
            tc.tile_pool(name="big", bufs=1) as big,
            tc.tile_pool(name="wts", bufs=1) as wts,
            tc.tile_pool(name="strips", bufs=2) as strips_p,
            tc.tile_pool(name="small", bufs=1) as small,
            tc.tile_pool(name="outp", bufs=4) as outp,
            tc.tile_pool(name="psA", bufs=1, space="PSUM") as psA,
            tc.tile_pool(name="psY", bufs=1, space="PSUM") as psY,
        ):
            xT = big.tile([128, EC, S], BF16, name="xT_sb")
            nc.sync.dma_start(xT[:, :, 0:IGW], xT_d.ap()[:, :, 0:IGW])
            for q in range(1, IG):
                nc.gpsimd.dma_start(
                    xT[:, :, q * IGW:(q + 1) * IGW],
                    xT_d.ap()[:, :, q * IGW:(q + 1) * IGW],
                )
            uT = big.tile([128, EC, S], BF16, name="uT_sb")
            v_sb = big.tile([128, JT, VW], BF16, name="v_sb")
            acc = [
                big.tile([128, E], ODT, name=f"acc_{gq}") for gq in range(QC)
            ]
            # ones column -> in-matmul softmax denominator
            nc.vector.memset(v_sb[:, :, E:E + 1], 1.0)
            if skip_p12:
                # ablation: P3 reads uT/v_sb which P1/P2 won't write
                # (walrus rejects memset on f32r; copy from zeroed f32)
                nc.vector.memset(uT[:], 0.0)
                nc.vector.memset(v_sb[:, :, 0:E], 0.0)

            import contextlib
            loop_cm = tc.For_i(0, loop, 1) if loop else contextlib.nullcontext()
            with loop_cm:
              for h in range(hpc):
                  beta_sb = wts.tile([128, JT], F32, tag="beta", bufs=2,
                                     name=f"beta_{h}")
                  nc.sync.dma_start(beta_sb[:], beta_d.ap()[h])

                  # ---- P1 (uT) and P2 (v) interleaved ----
                  a_sl = {}
                  c_sl = {}

                  def load_a(fc, h=h):
                      t = wts.tile([128, EC, 128], BF16, tag="a_sl", bufs=7,
                                   name=f"a_{h}_{fc}")
                      if tiny_in:
                          nc.sync.dma_start(t[:, :, 0:8], A_d.ap()[h][fc][:, :, 0:8])
                      else:
                          nc.sync.dma_start(t[:], A_d.ap()[h][fc])
                      return t

                  def load_c(fg, h=h):
                      t = wts.tile([128, EC, FH], BF16, tag="c_sl", bufs=3,
                                   name=f"c_{h}_{fg}")
                      if tiny_in:
                          nc.sync.dma_start(t[:, :, 0:8], C_d.ap()[h][fg][:, :, 0:8])
                      else:
                          nc.sync.dma_start(t[:], C_d.ap()[h][fg])
                      return t

                  if not skip_p12:
                      for fc in range(EC):
                          a_sl[fc] = load_a(fc)
                      c_sl[0] = load_c(0)
                      c_sl[1] = load_c(1)

                  def pu_group(k, h=h):
                      fc, jg = divmod(k, IG)
                      pu = psA.tile([128, IGW], F32, tag="a", bufs=psa_bufs,
                                    name=f"pu_{h}_{k}")
                      jsl = slice(jg * IGW, (jg + 1) * IGW)
                      for ec in range(EC):
                          nc.tensor.matmul(
                              pu[:], a_sl[fc][:, ec, :], xT[:, ec, jsl],
                              start=(ec == 0), stop=(ec == EC - 1),
                          )
                      nc.scalar.activation(uT[:, fc, jsl], pu[:], COPY)

                  def pv_group(k, h=h):
                      jt, fg = divmod(k, 2)
                      pv = psA.tile([128, IGW], F32, tag="a", bufs=psa_bufs,
                                    name=f"pv_{h}_{k}")
                      for ec in range(EC):
                          nc.tensor.matmul(
                              pv[:, 0:FH], xT[:, ec, jt * 128:(jt + 1) * 128],
                              c_sl[fg][:, ec, :],
                              start=(ec == 0), stop=(ec == EC - 1),
                          )
                      nc.vector.tensor_copy(
                          v_sb[:, jt, fg * FH:(fg + 1) * FH], pv[:, 0:FH])

                  # 24 pu groups, 32 pv groups: emit [pu pv pu pv pu pv pv] x 8
                  for blk in range(0 if skip_p12 else 8):
                      pu_group(blk * 3 + 0)
                      pv_group(blk * 4 + 0)
                      pu_group(blk * 3 + 1)
                      pv_group(blk * 4 + 1)
                      pu_group(blk * 3 + 2)
                      pv_group(blk * 4 + 2)
                      pv_group(blk * 4 + 3)

                  if skip_p3:
                      continue

                  # ---- P3: per query group, scores then transposed PV ----
                  for ig in range(IG):
                      isl = slice(ig * IGW, (ig + 1) * IGW)
                      sts = []
                      for jt in range(JT):
                          psc = psA.tile([128, IGW], F32, tag="a", bufs=psa_bufs,
                                         name=f"ps_{h}_{ig}_{jt}")
                          for fc in range(EC):
                              nc.tensor.matmul(
                                  psc[:], uT[:, fc, jt * 128:(jt + 1) * 128],
                                  xT[:, fc, isl],
                                  start=(fc == 0), stop=(fc == EC - 1),
                              )
                          st = strips_p.tile([128, IGW], BF16, tag="s", bufs=20,
                                             name=f"st_{h}_{ig}_{jt}")
                          nc.scalar.activation(
                              st[:], psc[:], EXP,
                              bias=beta_sb[:, jt:jt + 1], scale=0.125,
                          )
                          sts.append(st)
                      for qc in range(IGW // 128):
                          gq = ig * (IGW // 128) + qc
                          qsl = slice(qc * 128, (qc + 1) * 128)
                          yqd = psY.tile([128, VW], F32, tag="y", bufs=psy_bufs,
                                         name=f"y_{h}_{gq}")
                          for jt in range(JT):
                              nc.tensor.matmul(
                                  yqd[:, 0:512], sts[jt][:, qsl],
                                  v_sb[:, jt, 0:512],
                                  start=(jt == 0), stop=(jt == JT - 1),
                              )
                          for jt in range(JT):
                              nc.tensor.matmul(
                                  yqd[:, 512:E + 1], sts[jt][:, qsl],
                                  v_sb[:, jt, 512:E + 1],
                                  start=(jt == 0), stop=(jt == JT - 1),
                              )
                          rcb = small.tile([128, 1], F32, tag="rcb", bufs=4,
                                           name=f"rcb_{h}_{gq}")
                          nc.vector.reciprocal(rcb[:], yqd[:, E:E + 1])
                          if h == 0:
                              # first head writes the accumulator directly
                              nc.scalar.activation(acc[gq][:], yqd[:, 0:E],
                                                   COPY, scale=rcb[:, 0:1])
                          else:
                              ot = outp.tile([128, E], ODT, tag="ot", bufs=4,
                                             name=f"ot_{h}_{gq}")
                              nc.scalar.activation(ot[:], yqd[:, 0:E], COPY,
                                                   scale=rcb[:, 0:1])
                              nc.vector.tensor_tensor(
                                  acc[gq][:], acc[gq][:], ot[:], op=ADD)
                          if h == hpc - 1:
                              if tiny_out:
                                  nc.gpsimd.dma_start(
                                      out_d.ap()[gq][:, 0:8], acc[gq][:, 0:8])
                              else:
                                  nc.gpsimd.dma_start(
                                      out_d.ap()[gq], acc[gq][:])

    nc.compile()
    return nc


def _get_nc():
    global _CACHED_NC
    if _CACHED_NC is None:
        _CACHED_NC = _build_nc()
    return _CACHED_NC


def _prepare_inputs(x, Wq, Wk, Wv, bq, bk, bv, W0, b0):
    x = np.asarray(x, dtype=np.float32)
    Wq = np.asarray(Wq, dtype=np.float32)
    Wk = np.asarray(Wk, dtype=np.float32)
    Wv = np.asarray(Wv, dtype=np.float32)
    bq = np.asarray(bq, dtype=np.float32)
    bk = np.asarray(bk, dtype=np.float32)
    bv = np.asarray(bv, dtype=np.float32)
    W0 = np.asarray(W0, dtype=np.float32)
    b0 = np.asarray(b0, dtype=np.float32)

    # Per-head host precomputation (shared across batches)
    A = np.einsum("hem,hfm->hef", Wk, Wq)              # [H, E, E] = Wk @ Wq^T
    W0h = W0.reshape(H, E, E)                          # [H, E(f), E(n)]
    C = np.einsum("hef,hfn->hen", Wv, W0h)             # [H, E, E] = Wv @ W0
    wbeta = np.einsum("hef,hf->he", Wk, bq)            # [H, E]
    b_eff = b0 + np.einsum("he,hen->n", bv, W0h)       # [E]

    import ml_dtypes
    BF = ml_dtypes.bfloat16
    # contiguous per-slice layouts: A [H, EC, 128, EC, 128], C [H, 2, 128, EC, FH]
    A_l = np.stack([
        np.stack([_chunked(A[h])[:, :, fc * 128:(fc + 1) * 128] for fc in range(EC)])
        for h in range(H)
    ]).astype(BF)
    C_l = np.stack([
        np.stack([_chunked(C[h])[:, :, fg * FH:(fg + 1) * FH] for fg in range(2)])
        for h in range(H)
    ]).astype(BF)

    in_maps = []
    for c in range(8):
        b, hg = divmod(c, 2)
        hs = hg * HPC
        xT = _chunked(x[b].T).astype(BF)               # [128, EC, S]
        beta8 = np.einsum("se,he->hs", x[b], wbeta[hs:hs + HPC]) / 8.0
        beta8 = np.ascontiguousarray(
            beta8.reshape(HPC, JT, 128).transpose(0, 2, 1), dtype=np.float32
        )                                              # [HPC, 128, JT]
        in_maps.append({
            "xT": xT,
            "A": np.ascontiguousarray(A_l[hs:hs + HPC]),
            "C": np.ascontiguousarray(C_l[hs:hs + HPC]),
            "beta8": beta8,
        })
    return in_maps, b_eff


def _unshard(results, b_eff):
    out = np.zeros((B, S, E), dtype=np.float32)
    for c in range(8):
        o = np.asarray(results[c]["out"], dtype=np.float32)  # [QC, 128, E]
        out[c // 2] += o.reshape(S, E)
    out += b_eff[None, None, :]
    return out


def kernel(x, Wq, Wk, Wv, bq, bk, bv, W0, b0, _return_results=False):
    in_maps, b_eff = _prepare_inputs(x, Wq, Wk, Wv, bq, bk, bv, W0, b0)
    nc = _get_nc()
    res = run_bass_kernel_spmd(nc, in_maps, core_ids=list(range(8)))
    out = _unshard(res.results, b_eff)
    if _return_results:
        return out, res
    return out


# revision 17
# speedup vs baseline: 1.0386x; 1.0386x over previous
"""TRN2 Bass kernel for nn_MultiHeadAttention_42511586296095.

Reference math (B=4, S=2048, E=768, H=12, full-width per-head projections):
    q_h = x @ Wq_h + bq_h ; k_h = x @ Wk_h + bk_h ; v_h = x @ Wv_h + bv_h
    attn_h = softmax(q_h k_h^T / 8)
    out = sum_h (attn_h v_h) @ W0_h + b0

Sharding: 8 cores = 4 batches x 2 head-groups (6 heads each). Host sums the
two per-batch partials (device already accumulated over its own heads via
DRAM accum-DMA) and adds constants.

v3: algebraic folds of v2 (A = Wk Wq^T so scoresT = (x A) x^T with a
per-key bias beta; C = Wv W0 so the output projection disappears) plus a
transposed PV phase:

    P1: uT[f,j] = sum_e A[e,f] xT[e,j]            (keys j on free dim)
    P2: v[j,f]  = x C  -> SBUF bf16 [j, 772] with col 768 = 1.0
    P3 per 512-query group:
        scores:  16 chains psc = uT_jt^T xT_ig -> ACT exp (scale 1/8,
                 bias beta/8) -> bf16 strip st[key, query]  (2 PSUM banks,
                 double buffered)
        PV (transposed): per 128-query chunk qc, stationary = st[:, qc],
                 moving = v rows: yqd[q, 0:769] += st_chunk^T v_jt over all
                 16 key tiles.  Column 768 of v is 1.0, so yqd[:, 768] is
                 the softmax denominator -- no separate ones matmul.
        drain:   DVE reciprocal rcb = 1/yqd[:,768:769]; ACT Copy with
                 per-partition scale=rcb writes the normalized [128, 768]
                 bf16 output chunk; accum-DMA into DRAM across heads
                 (bypass on head 0).  Output layout is natural [S, E] --
                 the host just reshapes and upcasts, no transpose.

PSUM: tag "a" (P1/P2 chains + scores) 3 banks, tag "y" (yqd [128,772] =
2 banks each) x2 = 4 banks; 7 of 8 banks used (a3y2 measured best).  The
PV accumulation target is double buffered so query-chunk boundaries never
stall the PE, and scores chains only ever wait on an ACT drain three
chains back.

v4: ALL matmuls bf16 (f32 PSUM accumulation).  Same cycle count as f32r
(1 col/cycle, free dims 512/384/512/257), but ~half the PE datapath
energy and half the A/C weight-DMA traffic (28->14 MB/iter).  That
matters because the sustained clock is power-throttled: a P3-only
ablation (bf16-heavy) ran at the full 2.4 GHz while the f32r-heavy full
kernel sat at ~2.0 GHz.  Back-to-back, bf16 measures ~10-15% faster than
the f32r build at matched thermal state.  Numerics: bf16 operand noise
adds ~0.4% attention-weight jitter; measured rel err 8.8e-3 vs the 2e-2
gate.

Calibration (microbench, sustained 8-core): PE effective clock is
power-state dependent (2.0-2.4 GHz) + ~10-18 ns/matmul issue overhead;
matmul free dim is hard-capped at 512 (walrus s3d3_mm_num_elements, any
dtype, 3D APs too).  Stream floor 3.245M PE cycles -> 1.35 ms @2.4GHz;
this kernel measures ~1.68 ms (cool) to ~2.0 ms (throttled).
"""

import numpy as np

import concourse.bass as bass
import concourse.mybir as mybir
import concourse.tile as tile
from concourse import bacc
from concourse.bass_utils import run_bass_kernel_spmd

F32 = mybir.dt.float32
F32R = mybir.dt.float32r
BF16 = mybir.dt.bfloat16
EXP = mybir.ActivationFunctionType.Exp
COPY = mybir.ActivationFunctionType.Copy
ADD = mybir.AluOpType.add

B, S, E, H = 4, 2048, 768, 12
HPC = 6          # heads per core
EC = E // 128    # 6 chunks of the feature dim
JT = S // 128    # 16 key tiles
IG = 4           # query groups
IGW = S // IG    # 512 queries per group
FH = 384         # v-feature half width (2 halves of E)
QC = S // 128    # 16 query chunks (PV/output granularity)
VW = 772         # v_sb row width: 768 features + ones col at 768 + pad

_CACHED_NC = None


def _round_f32r(x: np.ndarray) -> np.ndarray:
    """Round fp32 to the hw f32r format: 11 explicit mantissa bits, RNE."""
    b = np.ascontiguousarray(x, dtype=np.float32).view(np.uint32).astype(np.uint64)
    shift = 12
    half = np.uint64(1 << (shift - 1))
    mask = np.uint64((1 << shift) - 1)
    r = (b + half) & ~mask
    tie = (b & mask) == half
    r[tie] = (b[tie] & ~mask) + (
        ((b[tie] >> np.uint64(shift)) & np.uint64(1)) << np.uint64(shift)
    )
    return r.astype(np.uint32).view(np.float32).reshape(x.shape)


def _chunked(a: np.ndarray) -> np.ndarray:
    """[E, N] -> SBUF layout [128, EC, N] with e = ec*128 + p."""
    ec = a.shape[0] // 128
    return np.ascontiguousarray(a.reshape(ec, 128, -1).transpose(1, 0, 2))


def _build_nc(hpc=HPC, loop=None, skip_p12=False, skip_p3=False, tiny_out=False, tiny_in=False, out_bf16=True, psa_bufs=3, psy_bufs=2):
    nc = bacc.Bacc("TRN2", target_bir_lowering=False, debug=False, num_devices=8)

    xT_d = nc.dram_tensor("xT", [128, EC, S], BF16, kind="ExternalInput")
    A_d = nc.dram_tensor("A", [HPC, EC, 128, EC, 128], BF16, kind="ExternalInput")
    C_d = nc.dram_tensor("C", [HPC, 2, 128, EC, FH], BF16, kind="ExternalInput")
    beta_d = nc.dram_tensor("beta8", [HPC, 128, JT], F32, kind="ExternalInput")
    ODT = BF16 if out_bf16 else F32
    out_d = nc.dram_tensor("out", [QC, 128, E], ODT, kind="ExternalOutput")

    out_chain = {}

    with tile.TileContext(nc) as tc:
        with (
            tc.tile_pool(name="big", bufs=1) as big,
            tc.tile_pool(name="wts", bufs=1) as wts,
            tc.tile_pool(name="strips", bufs=2) as strips_p,
            tc.tile_pool(name="small", bufs=1) as small,
            tc.tile_pool(name="outp", bufs=4) as outp,
            tc.tile_pool(name="psA", bufs=1, space="PSUM") as psA,
            tc.tile_pool(name="psY", bufs=1, space="PSUM") as psY,
        ):
            xT = big.tile([128, EC, S], BF16, name="xT_sb")
            nc.sync.dma_start(xT[:, :, 0:IGW], xT_d.ap()[:, :, 0:IGW])
            for q in range(1, IG):
                nc.gpsimd.dma_start(
                    xT[:, :, q * IGW:(q + 1) * IGW],
                    xT_d.ap()[:, :, q * IGW:(q + 1) * IGW],
                )
            uT = big.tile([128, EC, S], BF16, name="uT_sb")
            v_sb = big.tile([128, JT, VW], BF16, name="v_sb")
            acc = [
                big.tile([128, E], ODT, name=f"acc_{gq}") for gq in range(QC)
            ]
            # ones column -> in-matmul softmax denominator
            nc.vector.memset(v_sb[:, :, E:E + 1], 1.0)
            if skip_p12:
                # ablation: P3 reads uT/v_sb which P1/P2 won't write
                # (walrus rejects memset on f32r; copy from zeroed f32)
                nc.vector.memset(uT[:], 0.0)
                nc.vector.memset(v_sb[:, :, 0:E], 0.0)

            import contextlib
            loop_cm = tc.For_i(0, loop, 1) if loop else contextlib.nullcontext()
            with loop_cm:
              for h in range(hpc):
                  beta_sb = wts.tile([128, JT], F32, tag="beta", bufs=2,
                                     name=f"beta_{h}")
                  nc.sync.dma_start(beta_sb[:], beta_d.ap()[h])

                  # ---- P1 (uT) and P2 (v) interleaved ----
                  a_sl = {}
                  c_sl = {}

                  def load_a(fc, h=h):
                      t = wts.tile([128, EC, 128], BF16, tag="a_sl", bufs=7,
                                   name=f"a_{h}_{fc}")
                      if tiny_in:
                          nc.sync.dma_start(t[:, :, 0:8], A_d.ap()[h][fc][:, :, 0:8])
                      else:
                          nc.sync.dma_start(t[:], A_d.ap()[h][fc])
                      return t

                  def load_c(fg, h=h):
                      t = wts.tile([128, EC, FH], BF16, tag="c_sl", bufs=3,
                                   name=f"c_{h}_{fg}")
                      if tiny_in:
                          nc.sync.dma_start(t[:, :, 0:8], C_d.ap()[h][fg][:, :, 0:8])
                      else:
                          nc.sync.dma_start(t[:], C_d.ap()[h][fg])
                      return t

                  if not skip_p12:
                      for fc in range(EC):
                          a_sl[fc] = load_a(fc)
                      c_sl[0] = load_c(0)
                      c_sl[1] = load_c(1)

                  def pu_group(k, h=h):
                      fc, jg = divmod(k, IG)
                      pu = psA.tile([128, IGW], F32, tag="a", bufs=psa_bufs,
                                    name=f"pu_{h}_{k}")
                      jsl = slice(jg * IGW, (jg + 1) * IGW)
                      for ec in range(EC):
                          nc.tensor.matmul(
                              pu[:], a_sl[fc][:, ec, :], xT[:, ec, jsl],
                              start=(ec == 0), stop=(ec == EC - 1),
                          )
                      nc.scalar.activation(uT[:, fc, jsl], pu[:], COPY)

                  def pv_group(k, h=h):
                      jt, fg = divmod(k, 2)
                      pv = psA.tile([128, IGW], F32, tag="a", bufs=psa_bufs,
                                    name=f"pv_{h}_{k}")
                      for ec in range(EC):
                          nc.tensor.matmul(
                              pv[:, 0:FH], xT[:, ec, jt * 128:(jt + 1) * 128],
                              c_sl[fg][:, ec, :],
                              start=(ec == 0), stop=(ec == EC - 1),
                          )
                      nc.vector.tensor_copy(
                          v_sb[:, jt, fg * FH:(fg + 1) * FH], pv[:, 0:FH])

                  # 24 pu groups, 32 pv groups: emit [pu pv pu pv pu pv pv] x 8
                  for blk in range(0 if skip_p12 else 8):
                      pu_group(blk * 3 + 0)
                      pv_group(blk * 4 + 0)
                      pu_group(blk * 3 + 1)
                      pv_group(blk * 4 + 1)
                      pu_group(blk * 3 + 2)
                      pv_group(blk * 4 + 2)
                      pv_group(blk * 4 + 3)

                  if skip_p3:
                      continue

                  # ---- P3: per query group, scores then transposed PV ----
                  for ig in range(IG):
                      isl = slice(ig * IGW, (ig + 1) * IGW)
                      sts = []
                      for jt in range(JT):
                          psc = psA.tile([128, IGW], F32, tag="a", bufs=psa_bufs,
                                         name=f"ps_{h}_{ig}_{jt}")
                          for fc in range(EC):
                              nc.tensor.matmul(
                                  psc[:], uT[:, fc, jt * 128:(jt + 1) * 128],
                                  xT[:, fc, isl],
                                  start=(fc == 0), stop=(fc == EC - 1),
                              )
                          st = strips_p.tile([128, IGW], BF16, tag="s", bufs=20,
                                             name=f"st_{h}_{ig}_{jt}")
                          nc.scalar.activation(
                              st[:], psc[:], EXP,
                              bias=beta_sb[:, jt:jt + 1], scale=0.125,
                          )
                          sts.append(st)
                      for qc in range(IGW // 128):
                          gq = ig * (IGW // 128) + qc
                          qsl = slice(qc * 128, (qc + 1) * 128)
                          yqd = psY.tile([128, VW], F32, tag="y", bufs=psy_bufs,
                                         name=f"y_{h}_{gq}")
                          for jt in range(JT):
                              nc.tensor.matmul(
                                  yqd[:, 0:512], sts[jt][:, qsl],
                                  v_sb[:, jt, 0:512],
                                  start=(jt == 0), stop=(jt == JT - 1),
                              )
                          for jt in range(JT):
                              nc.tensor.matmul(
                                  yqd[:, 512:E + 1], sts[jt][:, qsl],
                                  v_sb[:, jt, 512:E + 1],
                                  start=(jt == 0), stop=(jt == JT - 1),
                              )
                          rcb = small.tile([128, 1], F32, tag="rcb", bufs=4,
                                           name=f"rcb_{h}_{gq}")
                          nc.vector.reciprocal(rcb[:], yqd[:, E:E + 1])
                          if h == 0:
                              # first head writes the accumulator directly
                              nc.scalar.activation(acc[gq][:], yqd[:, 0:E],
                                                   COPY, scale=rcb[:, 0:1])
                          else:
                              ot = outp.tile([128, E], ODT, tag="ot", bufs=4,
                                             name=f"ot_{h}_{gq}")
                              nc.scalar.activation(ot[:], yqd[:, 0:E], COPY,
                                                   scale=rcb[:, 0:1])
                              nc.vector.tensor_tensor(
                                  acc[gq][:], acc[gq][:], ot[:], op=ADD)
                          if h == hpc - 1:
                              if tiny_out:
                                  nc.gpsimd.dma_start(
                                      out_d.ap()[gq][:, 0:8], acc[gq][:, 0:8])
                              else:
                                  nc.gpsimd.dma_start(
                                      out_d.ap()[gq], acc[gq][:])

    nc.compile()
    return nc


def _get_nc():
    global _CACHED_NC
    if _CACHED_NC is None:
        _CACHED_NC = _build_nc()
    return _CACHED_NC


def _prepare_inputs(x, Wq, Wk, Wv, bq, bk, bv, W0, b0):
    x = np.asarray(x, dtype=np.float32)
    Wq = np.asarray(Wq, dtype=np.float32)
    Wk = np.asarray(Wk, dtype=np.float32)
    Wv = np.asarray(Wv, dtype=np.float32)
    bq = np.asarray(bq, dtype=np.float32)
    bk = np.asarray(bk, dtype=np.float32)
    bv = np.asarray(bv, dtype=np.float32)
    W0 = np.asarray(W0, dtype=np.float32)
    b0 = np.asarray(b0, dtype=np.float32)

    # Per-head host precomputation (shared across batches)
    A = np.einsum("hem,hfm->hef", Wk, Wq)              # [H, E, E] = Wk @ Wq^T
    W0h = W0.reshape(H, E, E)                          # [H, E(f), E(n)]
    C = np.einsum("hef,hfn->hen", Wv, W0h)             # [H, E, E] = Wv @ W0
    wbeta = np.einsum("hef,hf->he", Wk, bq)            # [H, E]
    b_eff = b0 + np.einsum("he,hen->n", bv, W0h)       # [E]

    import ml_dtypes
    BF = ml_dtypes.bfloat16
    # contiguous per-slice layouts: A [H, EC, 128, EC, 128], C [H, 2, 128, EC, FH]
    A_l = np.stack([
        np.stack([_chunked(A[h])[:, :, fc * 128:(fc + 1) * 128] for fc in range(EC)])
        for h in range(H)
    ]).astype(BF)
    C_l = np.stack([
        np.stack([_chunked(C[h])[:, :, fg * FH:(fg + 1) * FH] for fg in range(2)])
        for h in range(H)
    ]).astype(BF)

    in_maps = []
    for c in range(8):
        b, hg = divmod(c, 2)
        hs = hg * HPC
        xT = _chunked(x[b].T).astype(BF)               # [128, EC, S]
        beta8 = np.einsum("se,he->hs", x[b], wbeta[hs:hs + HPC]) / 8.0
        beta8 = np.ascontiguousarray(
            beta8.reshape(HPC, JT, 128).transpose(0, 2, 1), dtype=np.float32
        )                                              # [HPC, 128, JT]
        in_maps.append({
            "xT": xT,
            "A": np.ascontiguousarray(A_l[hs:hs + HPC]),
            "C": np.ascontiguousarray(C_l[hs:hs + HPC]),
            "beta8": beta8,
        })
    return in_maps, b_eff


def _unshard(results, b_eff):
    out = np.zeros((B, S, E), dtype=np.float32)
    for c in range(8):
        o = np.asarray(results[c]["out"], dtype=np.float32)  # [QC, 128, E]
        out[c // 2] += o.reshape(S, E)
    out += b_eff[None, None, :]
    return out


def kernel(x, Wq, Wk, Wv, bq, bk, bv, W0, b0, _return_results=False):
    in_maps, b_eff = _prepare_inputs(x, Wq, Wk, Wv, bq, bk, bv, W0, b0)
    nc = _get_nc()
    res = run_bass_kernel_spmd(nc, in_maps, core_ids=list(range(8)))
    out = _unshard(res.results, b_eff)
    if _return_results:
        return out, res
    return out


# revision 20
# speedup vs baseline: 1.0435x; 1.0047x over previous
"""TRN2 Bass kernel for nn_MultiHeadAttention_42511586296095.

Reference math (B=4, S=2048, E=768, H=12, full-width per-head projections):
    q_h = x @ Wq_h + bq_h ; k_h = x @ Wk_h + bk_h ; v_h = x @ Wv_h + bv_h
    attn_h = softmax(q_h k_h^T / 8)
    out = sum_h (attn_h v_h) @ W0_h + b0

Sharding: 8 cores = 4 batches x 2 head-groups (6 heads each). Host sums the
two per-batch partials (device already accumulated over its own heads via
DRAM accum-DMA) and adds constants.

v3: algebraic folds of v2 (A = Wk Wq^T so scoresT = (x A) x^T with a
per-key bias beta; C = Wv W0 so the output projection disappears) plus a
transposed PV phase:

    P1: uT[f,j] = sum_e A[e,f] xT[e,j]            (keys j on free dim)
    P2: v[j,f]  = x C  -> SBUF bf16 [j, 772] with col 768 = 1.0
    P3 per 512-query group:
        scores:  16 chains psc = uT_jt^T xT_ig -> ACT exp (scale 1/8,
                 bias beta/8) -> bf16 strip st[key, query]  (2 PSUM banks,
                 double buffered)
        PV (transposed): per 128-query chunk qc, stationary = st[:, qc],
                 moving = v rows: yqd[q, 0:769] += st_chunk^T v_jt over all
                 16 key tiles.  Column 768 of v is 1.0, so yqd[:, 768] is
                 the softmax denominator -- no separate ones matmul.
        drain:   DVE reciprocal rcb = 1/yqd[:,768:769]; ACT Copy with
                 per-partition scale=rcb writes the normalized [128, 768]
                 bf16 chunk; heads accumulate in SBUF (acc tiles, DVE
                 adds; head 0 written by ACT directly) and ONE plain DMA
                 per chunk on the last head writes DRAM -- 38MB/iter of
                 accum-RMW DMA traffic becomes 3.1MB.  Output layout is
                 natural [S, E] -- the host just reshapes and upcasts.

PSUM: tag "a" (P1/P2 chains + scores) 2 banks, tag "y" (yqd [128,772] =
2 banks each) x3 = 6 banks; all 8 banks used (a2y3 won an ABBA test on
the bf16 kernel by ~24us over a3y2).  The PV accumulation target is
triple buffered so query-chunk boundaries never stall the PE, and scores
chains only ever wait on an ACT drain two chains back.

v4: ALL matmuls bf16 (f32 PSUM accumulation).  Same cycle count as f32r
(1 col/cycle, free dims 512/384/512/257), but ~half the PE datapath
energy and half the A/C weight-DMA traffic (28->14 MB/iter).  That
matters because the sustained clock is power-throttled: a P3-only
ablation (bf16-heavy) ran at the full 2.4 GHz while the f32r-heavy full
kernel sat at ~2.0 GHz.  Back-to-back, bf16 measures ~10-15% faster than
the f32r build at matched thermal state.  Numerics: bf16 operand noise
adds ~0.4% attention-weight jitter; measured rel err 8.8e-3 vs the 2e-2
gate.

Calibration (microbench, sustained 8-core): PE effective clock is
power-state dependent (2.0-2.4 GHz) + ~10-18 ns/matmul issue overhead;
matmul free dim is hard-capped at 512 (walrus s3d3_mm_num_elements, any
dtype, 3D APs too).  Stream floor 3.245M PE cycles -> 1.35 ms @2.4GHz;
this kernel measures ~1.68 ms (cool) to ~2.0 ms (throttled).
"""

import numpy as np

import concourse.bass as bass
import concourse.mybir as mybir
import concourse.tile as tile
from concourse import bacc
from concourse.bass_utils import run_bass_kernel_spmd

F32 = mybir.dt.float32
F32R = mybir.dt.float32r
BF16 = mybir.dt.bfloat16
EXP = mybir.ActivationFunctionType.Exp
COPY = mybir.ActivationFunctionType.Copy
ADD = mybir.AluOpType.add

B, S, E, H = 4, 2048, 768, 12
HPC = 6          # heads per core
EC = E // 128    # 6 chunks of the feature dim
JT = S // 128    # 16 key tiles
IG = 4           # query groups
IGW = S // IG    # 512 queries per group
FH = 384         # v-feature half width (2 halves of E)
QC = S // 128    # 16 query chunks (PV/output granularity)
VW = 772         # v_sb row width: 768 features + ones col at 768 + pad

_CACHED_NC = None


def _round_f32r(x: np.ndarray) -> np.ndarray:
    """Round fp32 to the hw f32r format: 11 explicit mantissa bits, RNE."""
    b = np.ascontiguousarray(x, dtype=np.float32).view(np.uint32).astype(np.uint64)
    shift = 12
    half = np.uint64(1 << (shift - 1))
    mask = np.uint64((1 << shift) - 1)
    r = (b + half) & ~mask
    tie = (b & mask) == half
    r[tie] = (b[tie] & ~mask) + (
        ((b[tie] >> np.uint64(shift)) & np.uint64(1)) << np.uint64(shift)
    )
    return r.astype(np.uint32).view(np.float32).reshape(x.shape)


def _chunked(a: np.ndarray) -> np.ndarray:
    """[E, N] -> SBUF layout [128, EC, N] with e = ec*128 + p."""
    ec = a.shape[0] // 128
    return np.ascontiguousarray(a.reshape(ec, 128, -1).transpose(1, 0, 2))


def _build_nc(hpc=HPC, loop=None, skip_p12=False, skip_p3=False, tiny_out=False, tiny_in=False, out_bf16=True, psa_bufs=2, psy_bufs=3):
    nc = bacc.Bacc("TRN2", target_bir_lowering=False, debug=False, num_devices=8)

    xT_d = nc.dram_tensor("xT", [128, EC, S], BF16, kind="ExternalInput")
    A_d = nc.dram_tensor("A", [HPC, EC, 128, EC, 128], BF16, kind="ExternalInput")
    C_d = nc.dram_tensor("C", [HPC, 2, 128, EC, FH], BF16, kind="ExternalInput")
    beta_d = nc.dram_tensor("beta8", [HPC, 128, JT], F32, kind="ExternalInput")
    ODT = BF16 if out_bf16 else F32
    out_d = nc.dram_tensor("out", [QC, 128, E], ODT, kind="ExternalOutput")

    out_chain = {}

    with tile.TileContext(nc) as tc:
        with (
            tc.tile_pool(name="big", bufs=1) as big,
            tc.tile_pool(name="wts", bufs=1) as wts,
            tc.tile_pool(name="strips", bufs=2) as strips_p,
            tc.tile_pool(name="small", bufs=1) as small,
            tc.tile_pool(name="outp", bufs=4) as outp,
            tc.tile_pool(name="psA", bufs=1, space="PSUM") as psA,
            tc.tile_pool(name="psY", bufs=1, space="PSUM") as psY,
        ):
            xT = big.tile([128, EC, S], BF16, name="xT_sb")
            nc.sync.dma_start(xT[:, :, 0:IGW], xT_d.ap()[:, :, 0:IGW])
            for q in range(1, IG):
                nc.gpsimd.dma_start(
                    xT[:, :, q * IGW:(q + 1) * IGW],
                    xT_d.ap()[:, :, q * IGW:(q + 1) * IGW],
                )
            uT = big.tile([128, EC, S], BF16, name="uT_sb")
            v_sb = big.tile([128, JT, VW], BF16, name="v_sb")
            acc = [
                big.tile([128, E], ODT, name=f"acc_{gq}") for gq in range(QC)
            ]
            # ones column -> in-matmul softmax denominator
            nc.vector.memset(v_sb[:, :, E:E + 1], 1.0)
            if skip_p12:
                # ablation: P3 reads uT/v_sb which P1/P2 won't write
                # (walrus rejects memset on f32r; copy from zeroed f32)
                nc.vector.memset(uT[:], 0.0)
                nc.vector.memset(v_sb[:, :, 0:E], 0.0)

            import contextlib
            loop_cm = tc.For_i(0, loop, 1) if loop else contextlib.nullcontext()
            with loop_cm:
              for h in range(hpc):
                  beta_sb = wts.tile([128, JT], F32, tag="beta", bufs=2,
                                     name=f"beta_{h}")
                  nc.sync.dma_start(beta_sb[:], beta_d.ap()[h])

                  # ---- P1 (uT) and P2 (v) interleaved ----
                  a_sl = {}
                  c_sl = {}

                  def load_a(fc, h=h):
                      t = wts.tile([128, EC, 128], BF16, tag="a_sl", bufs=7,
                                   name=f"a_{h}_{fc}")
                      if tiny_in:
                          nc.sync.dma_start(t[:, :, 0:8], A_d.ap()[h][fc][:, :, 0:8])
                      else:
                          nc.sync.dma_start(t[:], A_d.ap()[h][fc])
                      return t

                  def load_c(fg, h=h):
                      t = wts.tile([128, EC, FH], BF16, tag="c_sl", bufs=3,
                                   name=f"c_{h}_{fg}")
                      if tiny_in:
                          nc.sync.dma_start(t[:, :, 0:8], C_d.ap()[h][fg][:, :, 0:8])
                      else:
                          nc.sync.dma_start(t[:], C_d.ap()[h][fg])
                      return t

                  if not skip_p12:
                      for fc in range(EC):
                          a_sl[fc] = load_a(fc)
                      c_sl[0] = load_c(0)
                      c_sl[1] = load_c(1)

                  def pu_group(k, h=h):
                      fc, jg = divmod(k, IG)
                      pu = psA.tile([128, IGW], F32, tag="a", bufs=psa_bufs,
                                    name=f"pu_{h}_{k}")
                      jsl = slice(jg * IGW, (jg + 1) * IGW)
                      for ec in range(EC):
                          nc.tensor.matmul(
                              pu[:], a_sl[fc][:, ec, :], xT[:, ec, jsl],
                              start=(ec == 0), stop=(ec == EC - 1),
                          )
                      nc.scalar.activation(uT[:, fc, jsl], pu[:], COPY)

                  def pv_group(k, h=h):
                      jt, fg = divmod(k, 2)
                      pv = psA.tile([128, IGW], F32, tag="a", bufs=psa_bufs,
                                    name=f"pv_{h}_{k}")
                      for ec in range(EC):
                          nc.tensor.matmul(
                              pv[:, 0:FH], xT[:, ec, jt * 128:(jt + 1) * 128],
                              c_sl[fg][:, ec, :],
                              start=(ec == 0), stop=(ec == EC - 1),
                          )
                      nc.vector.tensor_copy(
                          v_sb[:, jt, fg * FH:(fg + 1) * FH], pv[:, 0:FH])

                  # 24 pu groups, 32 pv groups: emit [pu pv pu pv pu pv pv] x 8
                  for blk in range(0 if skip_p12 else 8):
                      pu_group(blk * 3 + 0)
                      pv_group(blk * 4 + 0)
                      pu_group(blk * 3 + 1)
                      pv_group(blk * 4 + 1)
                      pu_group(blk * 3 + 2)
                      pv_group(blk * 4 + 2)
                      pv_group(blk * 4 + 3)

                  if skip_p3:
                      continue

                  # ---- P3: per query group, scores then transposed PV ----
                  for ig in range(IG):
                      isl = slice(ig * IGW, (ig + 1) * IGW)
                      sts = []
                      for jt in range(JT):
                          psc = psA.tile([128, IGW], F32, tag="a", bufs=psa_bufs,
                                         name=f"ps_{h}_{ig}_{jt}")
                          for fc in range(EC):
                              nc.tensor.matmul(
                                  psc[:], uT[:, fc, jt * 128:(jt + 1) * 128],
                                  xT[:, fc, isl],
                                  start=(fc == 0), stop=(fc == EC - 1),
                              )
                          st = strips_p.tile([128, IGW], BF16, tag="s", bufs=20,
                                             name=f"st_{h}_{ig}_{jt}")
                          nc.scalar.activation(
                              st[:], psc[:], EXP,
                              bias=beta_sb[:, jt:jt + 1], scale=0.125,
                          )
                          sts.append(st)
                      for qc in range(IGW // 128):
                          gq = ig * (IGW // 128) + qc
                          qsl = slice(qc * 128, (qc + 1) * 128)
                          yqd = psY.tile([128, VW], F32, tag="y", bufs=psy_bufs,
                                         name=f"y_{h}_{gq}")
                          for jt in range(JT):
                              nc.tensor.matmul(
                                  yqd[:, 0:512], sts[jt][:, qsl],
                                  v_sb[:, jt, 0:512],
                                  start=(jt == 0), stop=(jt == JT - 1),
                              )
                          for jt in range(JT):
                              nc.tensor.matmul(
                                  yqd[:, 512:E + 1], sts[jt][:, qsl],
                                  v_sb[:, jt, 512:E + 1],
                                  start=(jt == 0), stop=(jt == JT - 1),
                              )
                          rcb = small.tile([128, 1], F32, tag="rcb", bufs=4,
                                           name=f"rcb_{h}_{gq}")
                          nc.vector.reciprocal(rcb[:], yqd[:, E:E + 1])
                          if h == 0:
                              # first head writes the accumulator directly
                              nc.scalar.activation(acc[gq][:], yqd[:, 0:E],
                                                   COPY, scale=rcb[:, 0:1])
                          else:
                              ot = outp.tile([128, E], ODT, tag="ot", bufs=4,
                                             name=f"ot_{h}_{gq}")
                              nc.scalar.activation(ot[:], yqd[:, 0:E], COPY,
                                                   scale=rcb[:, 0:1])
                              nc.vector.tensor_tensor(
                                  acc[gq][:], acc[gq][:], ot[:], op=ADD)
                          if h == hpc - 1:
                              if tiny_out:
                                  nc.gpsimd.dma_start(
                                      out_d.ap()[gq][:, 0:8], acc[gq][:, 0:8])
                              else:
                                  nc.gpsimd.dma_start(
                                      out_d.ap()[gq], acc[gq][:])

    nc.compile()
    return nc


def _get_nc():
    global _CACHED_NC
    if _CACHED_NC is None:
        _CACHED_NC = _build_nc()
    return _CACHED_NC


def _prepare_inputs(x, Wq, Wk, Wv, bq, bk, bv, W0, b0):
    x = np.asarray(x, dtype=np.float32)
    Wq = np.asarray(Wq, dtype=np.float32)
    Wk = np.asarray(Wk, dtype=np.float32)
    Wv = np.asarray(Wv, dtype=np.float32)
    bq = np.asarray(bq, dtype=np.float32)
    bk = np.asarray(bk, dtype=np.float32)
    bv = np.asarray(bv, dtype=np.float32)
    W0 = np.asarray(W0, dtype=np.float32)
    b0 = np.asarray(b0, dtype=np.float32)

    # Per-head host precomputation (shared across batches)
    A = np.einsum("hem,hfm->hef", Wk, Wq)              # [H, E, E] = Wk @ Wq^T
    W0h = W0.reshape(H, E, E)                          # [H, E(f), E(n)]
    C = np.einsum("hef,hfn->hen", Wv, W0h)             # [H, E, E] = Wv @ W0
    wbeta = np.einsum("hef,hf->he", Wk, bq)            # [H, E]
    b_eff = b0 + np.einsum("he,hen->n", bv, W0h)       # [E]

    import ml_dtypes
    BF = ml_dtypes.bfloat16
    # contiguous per-slice layouts: A [H, EC, 128, EC, 128], C [H, 2, 128, EC, FH]
    A_l = np.stack([
        np.stack([_chunked(A[h])[:, :, fc * 128:(fc + 1) * 128] for fc in range(EC)])
        for h in range(H)
    ]).astype(BF)
    C_l = np.stack([
        np.stack([_chunked(C[h])[:, :, fg * FH:(fg + 1) * FH] for fg in range(2)])
        for h in range(H)
    ]).astype(BF)

    in_maps = []
    for c in range(8):
        b, hg = divmod(c, 2)
        hs = hg * HPC
        xT = _chunked(x[b].T).astype(BF)               # [128, EC, S]
        beta8 = np.einsum("se,he->hs", x[b], wbeta[hs:hs + HPC]) / 8.0
        beta8 = np.ascontiguousarray(
            beta8.reshape(HPC, JT, 128).transpose(0, 2, 1), dtype=np.float32
        )                                              # [HPC, 128, JT]
        in_maps.append({
            "xT": xT,
            "A": np.ascontiguousarray(A_l[hs:hs + HPC]),
            "C": np.ascontiguousarray(C_l[hs:hs + HPC]),
            "beta8": beta8,
        })
    return in_maps, b_eff


def _unshard(results, b_eff):
    out = np.zeros((B, S, E), dtype=np.float32)
    for c in range(8):
        o = np.asarray(results[c]["out"], dtype=np.float32)  # [QC, 128, E]
        out[c // 2] += o.reshape(S, E)
    out += b_eff[None, None, :]
    return out


def kernel(x, Wq, Wk, Wv, bq, bk, bv, W0, b0, _return_results=False):
    in_maps, b_eff = _prepare_inputs(x, Wq, Wk, Wv, bq, bk, bv, W0, b0)
    nc = _get_nc()
    res = run_bass_kernel_spmd(nc, in_maps, core_ids=list(range(8)))
    out = _unshard(res.results, b_eff)
    if _return_results:
        return out, res
    return out
